# revision 1
# baseline (speedup 1.0000x reference)
"""HGT kernel v1 for 8 trn2 NeuronCores — bf16 + host-sel + batched DMA.

Changes vs baseline:
  - bf16 data everywhere (xs, kv/q tables, collectives, scatters); fp32 only
    for PSUM, logits, reciprocal, and final y.
  - Host-built selection matrices (bf16) replace on-device transpose+is_equal.
  - Batched indirect DMA (BB tiles per gather/scatter) and batched DVE ops
    (VB tiles per instruction) to amortize fixed overheads.
  - Dense phases use DMA-transpose loads for lhsT (no PE transposes), bias via
    K=1 ones-row matmul, PSUM evacuation on ACT/DVE alternating.
  - Node counts padded to multiples of 128; phase D before phase C so the
    ReduceScatter overlaps phase-C edge tiles.
"""

import math
import os
import numpy as np
import ml_dtypes

try:
    import concourse  # noqa
except ImportError:
    import sys
    sys.path.insert(0, "/opt/trn_rl_repo")

from concourse import bacc, bass, mybir, tile
from concourse.bass import IndirectOffsetOnAxis
from concourse.bass_utils import run_bass_kernel_spmd

P = 128
H, DH, HID, IN_DIM, OUT_DIM = 8, 32, 256, 768, 128
L = 2
NU_F, NM_F, NR_F = 50000, 20000, 200000
C = 8
NU, NM, NR = NU_F // C, NM_F // C, NR_F // C          # 6250, 2500, 25000
NU_P, NM_P, NR_P = 6272, 2560, 25088                  # padded to %128
AG_BLK = NM_P + NU_P                                  # 8832 (movie first)
UBLK = 6400
BB = int(os.environ.get("KBB", "4"))   # tiles per indirect-DMA batch
VB = 4                                 # tiles per DVE instruction
F32 = mybir.dt.float32
BF = mybir.dt.bfloat16
I32 = mybir.dt.int32
AF = mybir.ActivationFunctionType
ALU = mybir.AluOpType
BF_NP = ml_dtypes.bfloat16

LAST_RESULTS = None


# ---------------------------------------------------------------- host prep

def _fold_weights(inp):
    """Fold a_rel/m_rel/p_rel into Wk/Wv per src type; sigmoid(skip) into Wa.
    All outputs cast to bf16 (biases as [1, w] rows for the ones-matmul)."""
    Wk, bk = inp["Wk"], inp["bk"]
    Wq, bq = inp["Wq"], inp["bq"]
    Wv, bv = inp["Wv"], inp["bv"]
    Wa, ba = inp["Wa"], inp["ba"]
    a_rel, m_rel, p_rel, skip = inp["a_rel"], inp["m_rel"], inp["p_rel"], inp["skip"]
    s_of_e = {0: 1, 1: 0, 2: 2}  # edge type -> src node type
    out = {}
    def b16(a):
        return np.ascontiguousarray(a).astype(BF_NP)
    for l in range(L):
        for e in range(3):
            s = s_of_e[e]
            wk_eff = np.empty((HID, HID), np.float32)
            bk_eff = np.empty((HID,), np.float32)
            wv_eff = np.empty((HID, HID), np.float32)
            bv_eff = np.empty((HID,), np.float32)
            for h in range(H):
                sl = slice(h * DH, (h + 1) * DH)
                sc = float(p_rel[l, e, h]) / math.sqrt(DH)
                wk_eff[:, sl] = (Wk[l, s][:, sl] @ a_rel[l, e, h]) * sc
                bk_eff[sl] = (bk[l, s][sl] @ a_rel[l, e, h]) * sc
                wv_eff[:, sl] = Wv[l, s][:, sl] @ m_rel[l, e, h]
                bv_eff[sl] = bv[l, s][sl] @ m_rel[l, e, h]
            out[f"wkv_t{s}_l{l}"] = b16(np.concatenate([wk_eff, wv_eff], 1))
            out[f"bkv_t{s}_l{l}"] = b16(np.concatenate([bk_eff, bv_eff]).reshape(1, 512))
        for t in (0, 2):
            out[f"wq_t{t}_l{l}"] = b16(Wq[l, t])
            out[f"bq_t{t}_l{l}"] = b16(np.asarray(bq[l, t]).reshape(1, HID))
        for t in range(3):
            g = 1.0 / (1.0 + math.exp(-float(skip[l, t])))
            out[f"omg_l{l}_t{t}"] = 1.0 - g
            if t != 1:
                out[f"wa_t{t}_l{l}"] = b16(np.asarray(Wa[l, t]) * g)
            out[f"ba_t{t}_l{l}"] = b16((np.asarray(ba[l, t]) * g).reshape(1, HID))
    out["w1"] = b16(inp["W1"])
    out["b1"] = b16(np.asarray(inp["b1"]).reshape(1, HID))
    out["w2"] = b16(inp["W2"])
    out["b2"] = b16(np.asarray(inp["b2"]).reshape(1, OUT_DIM))
    return out


def _pack(group_ids, payload_cols, pad_vals, dtypes):
    """Pack edges (sorted by group) into 128-slot tiles; groups never straddle
    a tile. Returns (T, [T,P] arrays)."""
    n = len(group_ids)
    if n == 0:
        return 0, [np.full((0, P), pv, dt) for pv, dt in zip(pad_vals, dtypes)]
    order = np.argsort(group_ids, kind="stable")
    g = group_ids[order]
    uniq, counts = np.unique(g, return_counts=True)
    ng = len(uniq)
    tile_id = np.empty(ng, np.int64)
    slot0 = np.empty(ng, np.int64)
    cur_t, fill = 0, 0
    cl = counts.tolist()
    for i in range(ng):
        c = cl[i]
        assert c <= P, f"group degree {c} > {P}"
        if fill + c > P:
            cur_t += 1
            fill = 0
        tile_id[i] = cur_t
        slot0[i] = fill
        fill += c
    T = cur_t + 1
    gi = np.repeat(np.arange(ng), counts)
    starts = np.cumsum(counts) - counts
    within = np.arange(n) - starts[gi]
    tid = tile_id[gi]
    slot = slot0[gi] + within
    outs = []
    for col, pv, dt in zip(payload_cols, pad_vals, dtypes):
        arr = np.full((T, P), pv, dtype=dt)
        arr[tid, slot] = col[order].astype(dt)
        outs.append(arr)
    return T, outs


def _sel_from_keys(keys):
    """keys: [T, P] int64 (pad slots must hold unique negatives).
    Returns [P, T*P] bf16 with tile t in cols [t*P,(t+1)*P)."""
    T = keys.shape[0]
    sel = (keys[:, :, None] == keys[:, None, :])
    return np.ascontiguousarray(
        sel.transpose(1, 0, 2).reshape(P, T * P).astype(BF_NP))


def _prep_edges(inp):
    """Per-core packed edge tiles + host-built selection matrices."""
    src_mr, dst_mr = inp["src_mr"], inp["dst_mr"]
    src_ur, dst_ur = inp["src_ur"], inp["dst_ur"]
    src_ru, dst_ru = inp["src_ru"], inp["dst_ru"]
    pad_slots = -1 - np.arange(P, dtype=np.int64)  # unique negative per slot

    # phase C: review-dst edges (mr type0 + ur type1), sharded by dst shard
    sm = (src_mr.astype(np.int64) // NM) * AG_BLK + (src_mr % NM)
    su = (src_ur.astype(np.int64) // NU) * AG_BLK + NM_P + (src_ur % NU)
    src_all = np.concatenate([sm, su])
    dst_all = np.concatenate([dst_mr, dst_ur]).astype(np.int64)
    typ_all = np.concatenate(
        [np.zeros(len(sm), np.int64), np.ones(len(su), np.int64)])
    csp = []
    pvC = [0, 0, NR_P, -1, -1]
    dtC = [np.int32, np.int32, np.int32, np.int64, np.int64]
    for c in range(C):
        m = (dst_all // NR) == c
        dl = dst_all[m] % NR
        cols = [src_all[m], dl, dl, dl * 2 + typ_all[m], dl]
        csp.append(_pack(dl, cols, pvC, dtC))
    T_C = max(max(t for t, _ in csp), 1)
    T_C = ((T_C + BB - 1) // BB) * BB
    cs = []
    for _, arrs in csp:
        padded = []
        for a, pv, dt in zip(arrs, pvC, dtC):
            full = np.full((T_C, P), pv, dtype=dt)
            full[: a.shape[0]] = a
            padded.append(full)
        # pad slots get unique negative keys (keys are cols 3,4)
        for ki in (3, 4):
            k = padded[ki]
            pad = k < 0
            k[pad] = np.broadcast_to(pad_slots, (T_C, P))[pad]
        cs.append({
            "src": np.ascontiguousarray(padded[0].T),
            "qi": np.ascontiguousarray(padded[1].T),
            "dst": np.ascontiguousarray(padded[2].T),
            "sden": _sel_from_keys(padded[3]),
            "sdst": _sel_from_keys(padded[4]),
        })

    # phase D: ru edges (review->user), sharded by src shard
    s64, d64 = src_ru.astype(np.int64), dst_ru.astype(np.int64)
    flat = (d64 // NU) * UBLK + (d64 % NU)
    qg = (d64 // NU) * NU_P + (d64 % NU)
    rup = []
    pvD = [0, 0, NU, -1]
    dtD = [np.int32, np.int32, np.int32, np.int64]
    for c in range(C):
        m = (s64 // NR) == c
        cols = [s64[m] % NR, qg[m], flat[m], flat[m]]
        rup.append(_pack(flat[m], cols, pvD, dtD))
    T_D = max(max(t for t, _ in rup), 1)
    T_D = ((T_D + BB - 1) // BB) * BB
    ru = []
    for _, arrs in rup:
        padded = []
        for a, pv, dt in zip(arrs, pvD, dtD):
            full = np.full((T_D, P), pv, dtype=dt)
            full[: a.shape[0]] = a
            padded.append(full)
        k = padded[3]
        pad = k < 0
        k[pad] = np.broadcast_to(pad_slots, (T_D, P))[pad]
        ru.append({
            "src": np.ascontiguousarray(padded[0].T),
            "qi": np.ascontiguousarray(padded[1].T),
            "dst": np.ascontiguousarray(padded[2].T),
            "sel": _sel_from_keys(padded[3]),
        })
    return T_C, cs, T_D, ru


# ---------------------------------------------------------------- device

def _strips(n, ss):
    return [(r0, min(ss, n - r0)) for r0 in range(0, n, ss)]


def build_program(T_C, T_D, omg):
    nc = bacc.Bacc("TRN2", target_bir_lowering=False, debug=False,
                   enable_asserts=False, num_devices=C)
    RG = [list(range(C))]

    def din(name, shape, dt=BF):
        return nc.dram_tensor(name, list(shape), dt, kind="ExternalInput")

    def dint(name, shape, dt=BF, shared=False):
        return nc.dram_tensor(name, list(shape), dt, kind="Internal",
                              addr_space="Shared" if shared else "Local")

    # inputs (host-transposed x, bf16)
    xT_u = din("xT_u", (IN_DIM, NU_P))
    xT_m = din("xT_m", (IN_DIM, NM_P))
    xT_r = din("xT_r", (IN_DIM, NR_P))
    cs_src = din("cs_src", (P, T_C), I32)
    cs_qi = din("cs_qi", (P, T_C), I32)
    cs_dst = din("cs_dst", (P, T_C), I32)
    cs_sden = din("cs_sden", (P, T_C * P), BF)
    cs_sdst = din("cs_sdst", (P, T_C * P), BF)
    ru_src = din("ru_src", (P, T_D), I32)
    ru_qi = din("ru_qi", (P, T_D), I32)
    ru_dst = din("ru_dst", (P, T_D), I32)
    ru_sel = din("ru_sel", (P, T_D * P), BF)
    w1 = din("w1", (IN_DIM, HID))
    b1 = din("b1", (1, HID))
    w2 = din("w2", (HID, OUT_DIM))
    b2 = din("b2", (1, OUT_DIM))
    wd, bd = {}, {}
    for l in range(L):
        for s in range(3):
            wd[f"wkv_t{s}_l{l}"] = din(f"wkv_t{s}_l{l}", (HID, 512))
            bd[f"bkv_t{s}_l{l}"] = din(f"bkv_t{s}_l{l}", (1, 512))
        for t in (0, 2):
            wd[f"wq_t{t}_l{l}"] = din(f"wq_t{t}_l{l}", (HID, HID))
            bd[f"bq_t{t}_l{l}"] = din(f"bq_t{t}_l{l}", (1, HID))
            wd[f"wa_t{t}_l{l}"] = din(f"wa_t{t}_l{l}", (HID, HID))
            bd[f"ba_t{t}_l{l}"] = din(f"ba_t{t}_l{l}", (1, HID))
        bd[f"ba_t1_l{l}"] = din(f"ba_t1_l{l}", (1, HID))
    y_u = nc.dram_tensor("y_u", [NU_P, OUT_DIM], F32, kind="ExternalOutput")
    y_m = nc.dram_tensor("y_m", [NM_P, OUT_DIM], F32, kind="ExternalOutput")
    y_r = nc.dram_tensor("y_r", [NR_P, OUT_DIM], F32, kind="ExternalOutput")

    # internal DRAM (bf16)
    xs = {t: [dint(f"xs_t{t}_s{s}", (n, HID)) for s in range(L + 1)]
          for t, n in ((0, NU_P), (1, NM_P), (2, NR_P))}
    kv_own = [dint(f"kv_own_l{l}", (AG_BLK, 512)) for l in range(L)]
    qu_own = [dint(f"qu_own_l{l}", (NU_P, HID)) for l in range(L)]
    kv_src = [dint(f"kv_src_l{l}", (C * AG_BLK, 512), shared=True) for l in range(L)]
    q_uf = [dint(f"q_uf_l{l}", (C * NU_P, HID), shared=True) for l in range(L)]
    kv_ru = [dint(f"kv_ru_l{l}", (NR_P, 512)) for l in range(L)]
    q_r = [dint(f"q_r_l{l}", (NR_P, HID)) for l in range(L)]
    outs_r = [dint(f"outs_r_l{l}", (NR_P + P, HID)) for l in range(L)]
    part_u = [dint(f"part_u_l{l}", (C * UBLK, 264)) for l in range(L)]
    red_u = [dint(f"red_u_l{l}", (UBLK, 264)) for l in range(L)]

    with tile.TileContext(nc) as tc:
        from contextlib import ExitStack
        _stk = ExitStack()
        wp = _stk.enter_context(tc.tile_pool(name="wp", bufs=1))

        def mk(shape, dt, name):
            return wp.tile(shape, dt, tag=name, name=name)

        ones = mk([1, P], BF, "ones")
        nc.vector.memset(ones[:], 1.0)

        # persistent SBUF: edge indices
        def persist(dr, shape, dt, name):
            t_ = mk(shape, dt, name)
            nc.sync.dma_start(t_[:], dr.ap()[:, :])
            return t_

        csrc_sb = persist(cs_src, [P, T_C], I32, "csrc")
        cqi_sb = persist(cs_qi, [P, T_C], I32, "cqi")
        cdst_sb = persist(cs_dst, [P, T_C], I32, "cdst")
        rsrc_sb = persist(ru_src, [P, T_D], I32, "rsrc")
        rqi_sb = persist(ru_qi, [P, T_D], I32, "rqi")
        rdst_sb = persist(ru_dst, [P, T_D], I32, "rdst")

        def load_w(dr, in_dim, out_w, name):
            ts = []
            for cch in range(in_dim // P):
                t_ = mk([P, out_w], BF, f"{name}_c{cch}")
                nc.sync.dma_start(t_[:], dr.ap()[cch * P:(cch + 1) * P, :])
                ts.append(t_)
            return ts

        def load_b(dr, w, name):
            t_ = mk([1, w], BF, name)
            nc.sync.dma_start(t_[0:1, :], dr.ap()[0:1, :])
            return t_

        w1_s = load_w(w1, IN_DIM, HID, "w1s")
        b1_s = load_b(b1, HID, "b1s")
        w2_s = load_w(w2, HID, OUT_DIM, "w2s")
        b2_s = load_b(b2, OUT_DIM, "b2s")
        ws, bs = {}, {}
        for k, dr in wd.items():
            ws[k] = load_w(dr, HID, 512 if k.startswith("wkv") else HID, k + "s")
        for k, dr in bd.items():
            bs[k] = load_b(dr, 512 if k.startswith("bkv") else HID, k + "s")

        sb = _stk.enter_context(tc.tile_pool(name="sb", bufs=2))
        se = _stk.enter_context(tc.tile_pool(name="se", bufs=3))
        pp = _stk.enter_context(tc.tile_pool(name="pp", bufs=2, space="PSUM"))

        zt = mk([P, 8, 264], BF, "zt")
        nc.vector.memset(zt[:], 0.0)

        def memset_dram(dr, nrows, w, tag):
            v = dr.ap()[0:nrows, :].rearrange("(a p) f -> p a f", p=P)
            a_tot = nrows // P
            a0 = 0
            while a0 < a_tot:
                aa = min(8, a_tot - a0)
                nc.sync.dma_start(v[:, a0:a0 + aa, :], zt[:, 0:aa, 0:w])
                a0 += aa

        # dense helper: out rows = act(x @ W + b); lhsT via DMA-transpose
        # loads (xs row-major) or direct strips (xT DRAM).
        SS = 512

        def dense(x_dr, n, in_dim, jobs, tag, x_is_T=False):
            # jobs: (w_tiles, bias_tile [1,w], out_w, finish(ot_strip, r0, na))
            nch = in_dim // P
            for si, (r0, ss) in enumerate(_strips(n, SS)):
                na = ss // P
                if x_is_T:
                    xT = sb.tile([P, nch, SS], BF, tag="dxTT")
                    v = x_dr.ap().rearrange("(c p) n -> p c n", p=P)
                    nc.sync.dma_start(xT[:, :, 0:ss], v[:, :, r0:r0 + ss])
                    chunks = [xT[:, c, 0:ss] for c in range(nch)]
                else:
                    chunks = []
                    for cch in range(nch):
                        xT = sb.tile([P, SS], BF, tag=f"dxT{cch}")
                        nc.sync.dma_start(
                            xT[:, 0:ss],
                            x_dr.ap()[r0:r0 + ss, cch * P:(cch + 1) * P],
                            transpose=True)
                        chunks.append(xT[:, 0:ss])
                outs = []
                for ji, (wt, bt, ow, finish) in enumerate(jobs):
                    ot = sb.tile([P, SS // P, ow], BF, tag=f"dot{ji}")
                    outs.append(ot)
                for a in range(na):
                    sl = slice(a * P, (a + 1) * P)
                    for ji, (wt, bt, ow, finish) in enumerate(jobs):
                        ps = pp.tile([P, 512], F32, tag="ps")
                        nc.tensor.matmul(out=ps[:, 0:ow], lhsT=ones[0:1, :],
                                         rhs=bt[0:1, :], start=True, stop=False)
                        for cch in range(nch):
                            nc.tensor.matmul(out=ps[:, 0:ow],
                                             lhsT=chunks[cch][:, sl],
                                             rhs=wt[cch][:], start=False,
                                             stop=(cch == nch - 1))
                        if (a + ji) % 2 == 0:
                            nc.scalar.activation(out=outs[ji][:, a, :],
                                                 in_=ps[:, 0:ow], func=AF.Copy)
                        else:
                            nc.vector.tensor_copy(outs[ji][:, a, :], ps[:, 0:ow])
                for ji, (wt, bt, ow, finish) in enumerate(jobs):
                    finish(outs[ji], r0, na)

        def fin_store(out_dr, off, ow, act=None, alpha=0.0, out_f32=False):
            def f(ot, r0, na):
                src = ot
                if act is not None or out_f32:
                    o2 = sb.tile([P, SS // P, ow], F32 if out_f32 else BF,
                                 tag="finact")
                    if act is not None:
                        nc.scalar.activation(out=o2[:, 0:na, :],
                                             in_=ot[:, 0:na, :], func=act,
                                             alpha=alpha)
                    else:
                        nc.vector.tensor_copy(o2[:, 0:na, :], ot[:, 0:na, :])
                    src = o2
                v = out_dr.ap()[off + r0: off + r0 + na * P, :].rearrange(
                    "(a p) f -> p a f", p=P)
                nc.sync.dma_start(v[:, :, :], src[:, 0:na, :])
            return f

        for l in range(L):
            memset_dram(outs_r[l], NR_P + P, HID, f"z1{l}")
            memset_dram(part_u[l], C * UBLK, 264, f"z2{l}")

        # ---- phase 0: input MLP (reads host-transposed x)
        for t, x_dr, n in ((0, xT_u, NU_P), (1, xT_m, NM_P), (2, xT_r, NR_P)):
            dense(x_dr, n, IN_DIM,
                  [(w1_s, b1_s, HID,
                    fin_store(xs[t][0], 0, HID, act=AF.Lrelu, alpha=0.01))],
                  f"p0t{t}", x_is_T=True)

        for l in range(L):
            # ---- phase A: user + movie kqv, then AG, then review kqv
            dense(xs[0][l], NU_P, HID, [
                (ws[f"wkv_t0_l{l}"], bs[f"bkv_t0_l{l}"], 512,
                 fin_store(kv_own[l], NM_P, 512)),
                (ws[f"wq_t0_l{l}"], bs[f"bq_t0_l{l}"], HID,
                 fin_store(qu_own[l], 0, HID)),
            ], f"au{l}")
            dense(xs[1][l], NM_P, HID, [
                (ws[f"wkv_t1_l{l}"], bs[f"bkv_t1_l{l}"], 512,
                 fin_store(kv_own[l], 0, 512)),
            ], f"am{l}")
            nc.gpsimd.collective_compute(
                "AllGather", ALU.bypass, replica_groups=RG,
                ins=[kv_own[l].ap()], outs=[kv_src[l].ap()])
            nc.gpsimd.collective_compute(
                "AllGather", ALU.bypass, replica_groups=RG,
                ins=[qu_own[l].ap()], outs=[q_uf[l].ap()])
            dense(xs[2][l], NR_P, HID, [
                (ws[f"wkv_t2_l{l}"], bs[f"bkv_t2_l{l}"], 512,
                 fin_store(kv_ru[l], 0, 512)),
                (ws[f"wq_t2_l{l}"], bs[f"bq_t2_l{l}"], HID,
                 fin_store(q_r[l], 0, HID)),
            ], f"ar{l}")

            # ---- edge-phase helper (shared by C and D)
            def edge_batch(b0, kv_dr, q_dr, src_sb, qi_sb, sel_dr2, norm):
                kvg = se.tile([P, BB, 512], BF, tag="kvg")
                qg = se.tile([P, BB, HID], BF, tag="qg")
                for b in range(BB):
                    nc.gpsimd.indirect_dma_start(
                        out=kvg[:, b, :], out_offset=None, in_=kv_dr.ap(),
                        in_offset=IndirectOffsetOnAxis(
                            ap=src_sb[:, b0 + b:b0 + b + 1], axis=0))
                    nc.gpsimd.indirect_dma_start(
                        out=qg[:, b, :], out_offset=None, in_=q_dr.ap(),
                        in_offset=IndirectOffsetOnAxis(
                            ap=qi_sb[:, b0 + b:b0 + b + 1], axis=0))
                sels = []
                for sd, nm_ in sel_dr2:
                    s_ = se.tile([P, BB * P], BF, tag=nm_)
                    nc.sync.dma_start(s_[:], sd.ap()[:, b0 * P:(b0 + BB) * P])
                    sels.append(s_)
                ow = 256 if norm else 264
                mo = se.tile([P, BB, ow], BF, tag="moC" if norm else "moD")
                for v0 in range(0, BB, VB):
                    kq = se.tile([P, VB, HID], BF, tag="kq")
                    nc.vector.tensor_mul(kq[:], kvg[:, v0:v0 + VB, 0:HID],
                                         qg[:, v0:v0 + VB, :])
                    lg = se.tile([P, VB, H], F32, tag="lg")
                    nc.vector.tensor_reduce(
                        out=lg[:],
                        in_=kq[:].rearrange("p b (h d) -> p b h d", h=H),
                        axis=mybir.AxisListType.X, op=ALU.add)
                    if norm:
                        ex = se.tile([P, VB, H], BF, tag="ex")
                        nc.scalar.activation(out=ex[:], in_=lg[:], func=AF.Exp)
                        dps = pp.tile([P, VB, H], F32, tag="dps")
                        for t in range(VB):
                            j = v0 + t
                            nc.tensor.matmul(
                                out=dps[:, t, :],
                                lhsT=sels[0][:, j * P:(j + 1) * P],
                                rhs=ex[:, t, :], start=True, stop=True)
                        rden = se.tile([P, VB, H], F32, tag="rden")
                        nc.vector.reciprocal(out=rden[:], in_=dps[:])
                        attn = se.tile([P, VB, H], BF, tag="attn")
                        nc.vector.tensor_mul(attn[:], ex[:], rden[:])
                        wsrc, selm = attn, sels[1]
                        wv = se.tile([P, VB, 256], BF, tag="wv")
                        wv_out = wv[:]
                        vslice = slice(0, 256)
                    else:
                        rhs = se.tile([P, VB, 264], BF, tag="rhsD")
                        nc.scalar.activation(out=rhs[:, :, 0:H], in_=lg[:],
                                             func=AF.Exp)
                        wsrc, selm = rhs[:, :, 0:H], sels[0]
                        wv_out = rhs[:, :, H:264]
                        vslice = None
                    nc.vector.tensor_tensor(
                        out=wv_out.rearrange("p b (h d) -> p b h d", h=H),
                        in0=kvg[:, v0:v0 + VB, 256:512]
                            .rearrange("p b (h d) -> p b h d", h=H),
                        in1=wsrc.rearrange("p b (h o) -> p b h o", h=H)
                            .to_broadcast([P, VB, H, DH]),
                        op=ALU.mult)
                    for u0 in range(0, VB, 2):
                        mps = pp.tile([P, 2, 512], F32, tag="mops")
                        for tt in range(2):
                            j = v0 + u0 + tt
                            r_ = (wv[:, u0 + tt, :] if norm
                                  else rhs[:, u0 + tt, 0:264])
                            nc.tensor.matmul(
                                out=mps[:, tt, 0:ow],
                                lhsT=selm[:, j * P:(j + 1) * P],
                                rhs=r_, start=True, stop=True)
                        nc.scalar.activation(
                            out=mo[:, v0 + u0:v0 + u0 + 2, :],
                            in_=mps[:, :, 0:ow], func=AF.Copy)
                return mo

            # ---- phases C and D interleaved (independent given the AGs);
            # ReduceScatter issued right after the last D batch so it overlaps
            # the remaining C batches.
            nD, nC = T_D // BB, T_C // BB
            for i in range(max(nD, nC)):
                if i < nD:
                    b0 = i * BB
                    mo = edge_batch(b0, kv_ru[l], q_uf[l], rsrc_sb, rqi_sb,
                                    [(ru_sel, "selD")], norm=False)
                    for b in range(BB):
                        nc.gpsimd.indirect_dma_start(
                            out=part_u[l].ap(), in_=mo[:, b, :],
                            out_offset=IndirectOffsetOnAxis(
                                ap=rdst_sb[:, b0 + b:b0 + b + 1], axis=0),
                            in_offset=None)
                    if i == nD - 1:
                        nc.gpsimd.collective_compute(
                            "ReduceScatter", ALU.add, replica_groups=RG,
                            ins=[part_u[l].ap()], outs=[red_u[l].ap()])
                if i < nC:
                    b0 = i * BB
                    mo = edge_batch(b0, kv_src[l], q_r[l], csrc_sb, cqi_sb,
                                    [(cs_sden, "selCd"), (cs_sdst, "selCm")],
                                    norm=True)
                    for b in range(BB):
                        nc.gpsimd.indirect_dma_start(
                            out=outs_r[l].ap(), in_=mo[:, b, :],
                            out_offset=IndirectOffsetOnAxis(
                                ap=cdst_sb[:, b0 + b:b0 + b + 1], axis=0),
                            in_offset=None)

            # ---- phase E
            # reviews: att rows in outs_r; attT via DMA-transpose, gelu on ACT,
            # matmul with Wa, bias via ones-mm, blend with og*xs.
            og_r = omg[(l, 2)]
            wa_r = ws[f"wa_t2_l{l}"]
            ba_r = bs[f"ba_t2_l{l}"]
            for r0, ss in _strips(NR_P, SS):
                na = ss // P
                attT = []
                for cch in range(2):
                    tt_ = sb.tile([P, SS], BF, tag=f"eatt{cch}")
                    nc.sync.dma_start(
                        tt_[:, 0:ss],
                        outs_r[l].ap()[r0:r0 + ss, cch * P:(cch + 1) * P],
                        transpose=True)
                    nc.scalar.activation(out=tt_[:, 0:ss], in_=tt_[:, 0:ss],
                                         func=AF.Gelu)
                    attT.append(tt_)
                xg = sb.tile([P, SS // P, HID], BF, tag="exg")
                vv = xs[2][l].ap()[r0:r0 + ss, :].rearrange(
                    "(a p) f -> p a f", p=P)
                nc.sync.dma_start(xg[:, 0:na, :], vv)
                nc.vector.tensor_scalar_mul(out=xg[:, 0:na, :],
                                            in0=xg[:, 0:na, :], scalar1=og_r)
                ot = sb.tile([P, SS // P, HID], BF, tag="eot")
                for a in range(na):
                    sl = slice(a * P, (a + 1) * P)
                    ps = pp.tile([P, 512], F32, tag="ps")
                    nc.tensor.matmul(out=ps[:, 0:HID], lhsT=ones[0:1, :],
                                     rhs=ba_r[0:1, :], start=True, stop=False)
                    for cch in range(2):
                        nc.tensor.matmul(out=ps[:, 0:HID],
                                         lhsT=attT[cch][:, sl],
                                         rhs=wa_r[cch][:], start=False,
                                         stop=(cch == 1))
                    nc.scalar.activation(out=ot[:, a, :], in_=ps[:, 0:HID],
                                         func=AF.Copy)
                nc.vector.tensor_add(ot[:, 0:na, :], ot[:, 0:na, :],
                                     xg[:, 0:na, :])
                vv2 = xs[2][l + 1].ap()[r0:r0 + ss, :].rearrange(
                    "(a p) f -> p a f", p=P)
                nc.sync.dma_start(vv2, ot[:, 0:na, :])

            # users: normalize red_u rows, gelu, PE-transpose, matmul, blend
            og_u = omg[(l, 0)]
            wa_u = ws[f"wa_t0_l{l}"]
            ba_u = bs[f"ba_t0_l{l}"]
            ident = mk([P, P], BF, "identE") if l == 0 else ident
            if l == 0:
                from concourse.masks import make_identity
                make_identity(nc, ident[:, :])
            for a in range(NU_P // P):
                r0 = a * P
                rt = sb.tile([P, 264], BF, tag="eur")
                nc.sync.dma_start(rt[:], red_u[l].ap()[r0:r0 + P, :])
                rd = sb.tile([P, H], F32, tag="eurd")
                nc.vector.tensor_scalar_add(out=rd[:], in0=rt[:, 0:H],
                                            scalar1=1e-12)
                nc.vector.reciprocal(out=rd[:], in_=rd[:])
                at = sb.tile([P, HID], BF, tag="euat")
                nc.vector.tensor_tensor(
                    out=at[:].rearrange("p (h d) -> p h d", h=H),
                    in0=rt[:, H:264].rearrange("p (h d) -> p h d", h=H),
                    in1=rd[:].rearrange("p (h o) -> p h o", h=H)
                        .to_broadcast([P, H, DH]),
                    op=ALU.mult)
                nc.scalar.activation(out=at[:], in_=at[:], func=AF.Gelu)
                attT = []
                for cch in range(2):
                    tp = pp.tile([P, P], BF, tag="dps")
                    nc.tensor.transpose(out=tp[:], in_=at[:, cch * P:(cch + 1) * P],
                                        identity=ident[:, :])
                    ts_ = sb.tile([P, P], BF, tag=f"ets{cch}")
                    nc.vector.tensor_copy(ts_[:], tp[:])
                    attT.append(ts_)
                ps = pp.tile([P, 512], F32, tag="ps")
                nc.tensor.matmul(out=ps[:, 0:HID], lhsT=ones[0:1, :],
                                 rhs=ba_u[0:1, :], start=True, stop=False)
                for cch in range(2):
                    nc.tensor.matmul(out=ps[:, 0:HID], lhsT=attT[cch][:],
                                     rhs=wa_u[cch][:], start=False,
                                     stop=(cch == 1))
                ot = sb.tile([P, HID], BF, tag="euo")
                nc.scalar.activation(out=ot[:], in_=ps[:, 0:HID], func=AF.Copy)
                xg = sb.tile([P, HID], BF, tag="euxg")
                nc.sync.dma_start(xg[:], xs[0][l].ap()[r0:r0 + P, :])
                nc.vector.tensor_scalar_mul(out=xg[:], in0=xg[:], scalar1=og_u)
                nc.vector.tensor_add(ot[:], ot[:], xg[:])
                nc.sync.dma_start(xs[0][l + 1].ap()[r0:r0 + P, :], ot[:])

            # movies: new_x = og*x + g*ba
            og_m = omg[(l, 1)]
            bam = bs[f"ba_t1_l{l}"]
            bam_b = sb.tile([P, HID], BF, tag="embb")
            nc.sync.dma_start(bam_b[:],
                              bd[f"ba_t1_l{l}"].ap()[0:1, :].to_broadcast([P, HID]))
            for a in range(NM_P // P):
                r0 = a * P
                xg = sb.tile([P, HID], BF, tag="emx")
                nc.sync.dma_start(xg[:], xs[1][l].ap()[r0:r0 + P, :])
                nc.vector.tensor_scalar_mul(out=xg[:], in0=xg[:], scalar1=og_m)
                nc.vector.tensor_add(xg[:], xg[:], bam_b[:])
                nc.sync.dma_start(xs[1][l + 1].ap()[r0:r0 + P, :], xg[:])

        # ---- phase F: output MLP (fp32 out)
        for t, y_dr, n in ((0, y_u, NU_P), (1, y_m, NM_P), (2, y_r, NR_P)):
            dense(xs[t][L], n, HID,
                  [(w2_s, b2_s, OUT_DIM,
                    fin_store(y_dr, 0, OUT_DIM, act=AF.Lrelu, alpha=0.01,
                              out_f32=True))], f"pft{t}")
        _stk.close()

    nc.finalize()
    return nc


# ---------------------------------------------------------------- entry

_CACHE = {}


def kernel(**inputs):
    inp = {k: np.asarray(v) for k, v in inputs.items()}
    w = _fold_weights(inp)
    T_C, cs, T_D, ru = _prep_edges(inp)
    omg = {(l, t): w[f"omg_l{l}_t{t}"] for l in range(L) for t in range(3)}

    key = (T_C, T_D)
    if key not in _CACHE:
        _CACHE[key] = build_program(T_C, T_D, omg)
    nc = _CACHE[key]

    def xT_pad(x, n_r, n_p):
        out = np.zeros((IN_DIM, n_p), BF_NP)
        out[:, :n_r] = np.ascontiguousarray(x.T).astype(BF_NP)
        return out

    in_maps = []
    for c in range(C):
        m = {
            "xT_u": xT_pad(inp["x_user"][c * NU:(c + 1) * NU], NU, NU_P),
            "xT_m": xT_pad(inp["x_movie"][c * NM:(c + 1) * NM], NM, NM_P),
            "xT_r": xT_pad(inp["x_review"][c * NR:(c + 1) * NR], NR, NR_P),
            "w1": w["w1"], "b1": w["b1"], "w2": w["w2"], "b2": w["b2"],
            "cs_src": cs[c]["src"], "cs_qi": cs[c]["qi"],
            "cs_dst": cs[c]["dst"], "cs_sden": cs[c]["sden"],
            "cs_sdst": cs[c]["sdst"],
            "ru_src": ru[c]["src"], "ru_qi": ru[c]["qi"],
            "ru_dst": ru[c]["dst"], "ru_sel": ru[c]["sel"],
        }
        for l in range(L):
            for s in range(3):
                m[f"wkv_t{s}_l{l}"] = w[f"wkv_t{s}_l{l}"]
                m[f"bkv_t{s}_l{l}"] = w[f"bkv_t{s}_l{l}"]
            for t in (0, 2):
                for nme in (f"wq_t{t}_l{l}", f"bq_t{t}_l{l}",
                            f"wa_t{t}_l{l}", f"ba_t{t}_l{l}"):
                    m[nme] = w[nme]
            m[f"ba_t1_l{l}"] = w[f"ba_t1_l{l}"]
        in_maps.append(m)

    trace = os.environ.get("BASS_KERNEL_TRACE") == "1"
    res = run_bass_kernel_spmd(nc, in_maps, core_ids=list(range(C)),
                               trace=trace)
    global LAST_RESULTS
    LAST_RESULTS = res
    r = res.results
    yu = np.concatenate([np.asarray(r[c]["y_u"])[:NU] for c in range(C)], 0)
    ym = np.concatenate([np.asarray(r[c]["y_m"])[:NM] for c in range(C)], 0)
    yr = np.concatenate([np.asarray(r[c]["y_r"])[:NR] for c in range(C)], 0)
    return np.concatenate([yu, ym, yr], 0).astype(np.float32)

